# revision 8
# baseline (speedup 1.0000x reference)
"""CapsNet dynamic-routing layer on 8 Trainium2 NeuronCores (v2).

Math (per example, S=512 input capsules of dim D=256, 16 output capsules of
dim 32, O = 16*32 = 512):
  u_hat = x @ W                     # [S, O]
  b = 0; for 3 routing iters:
    c = softmax_n(b)                # over the 16-capsule axis
    s[n] = sum_s c[n,s] * u_hat[s, n*32:(n+1)*32]
    v = s / sqrt(|s|^2 + 1e-7)
    b[n,s] = v[n] . u_hat[s, n*32:(n+1)*32]
  out = v.flatten()

Sharding: pure data-parallel over the batch (256 examples -> 32 per core),
W replicated, no cross-core communication.

v2 design (vs baseline):
- uhT = (x@W).T computed ONCE via matmul (W stationary, reused), u_hat
  obtained from uhT by PE transposes (fp16, 1 cyc/row) - halves creation PE.
- Routing matmuls (s = c@u_hat and b = v@u_hatT) use PE COLUMN TILING:
  the 4 examples of a group run concurrently in 4 32-column PE groups
  (tile_position=(0,32j), fp16 operands so PSUM partition offsets are legal).
  This turns 4 serial N=512 streams into ~1.
- Iteration 0 is folded into creation: c0 is uniform=1/16, so
  s0 = colsum(u_hat)/16 comes free from accum_out on the uhT psum->sbuf
  copies; squash runs in O-partition layout via two tiny matmuls
  (32-partition-group norm + partition-group broadcast).
- Routing operands in fp16 (~5e-4/elem); all accumulation fp32.
"""

import sys

sys.path.insert(0, "/opt/trn_rl_repo")

import numpy as np

import concourse.bacc as bacc
import concourse.mybir as mybir
import concourse.tile as tile
from concourse import bass
from concourse.bass_utils import run_bass_kernel_spmd

F32 = mybir.dt.float32
F32R = mybir.dt.float32r
F16 = mybir.dt.float16
U32 = mybir.dt.uint32
QMAGIC = 0x5F3759DF  # quake rsqrt seed
AX = mybir.AxisListType
AF = mybir.ActivationFunctionType
OP = mybir.AluOpType

B, S, D = 256, 512, 256
NC_, DC = 16, 32  # num_capsule, dim_capsule
O = NC_ * DC  # 512
N_CORES = 8
E = B // N_CORES  # 32 examples per core
G = 4  # examples per group (one per PE column-group)
KT_D = D // 128  # 2 k-tiles over D
MT = 4  # 4 tiles over S and over O


def host_consts():
    # dmask[32j+n, n'*32+d] = (n' == n) for n < 16, else 0
    dmask = np.zeros((128, O), np.float32)
    for j in range(G):
        for n in range(NC_):
            dmask[32 * j + n, n * DC : (n + 1) * DC] = 1.0
    # vmask[p, j*128 + k*32 + n'] = (n' == 4k + p//32), n' in [0,32)
    vmask = np.zeros((128, G * 4 * DC), np.float16)
    for p in range(128):
        for j in range(G):
            for k in range(4):
                vmask[p, j * 128 + k * DC + 4 * k + p // 32] = 1.0
    identH = np.eye(128, dtype=np.float16)
    identF = np.eye(128, dtype=np.float32)
    ind4 = np.zeros((128, 4), np.float32)
    for p in range(128):
        ind4[p, p // 32] = 1.0
    ind4T = np.ascontiguousarray(ind4.T)
    return dmask, vmask, identH, identF, ind4, ind4T


def quake_rsqrt(nc, sp, q, magic, P, N, tag):
    """y ~= rsqrt(q) via quake seed + 2 Newton steps (all DVE, no ACT table)."""
    sh = sp.tile([P, N], U32, tag=f"{tag}_sh")
    nc.vector.tensor_scalar(
        sh[:P, :], q.bitcast(U32), 1, None, op0=OP.logical_shift_right
    )
    y = sp.tile([P, N], F32, tag=f"{tag}_y")
    nc.vector.tensor_tensor(
        y[:P, :].bitcast(U32), magic[:P, :N], sh[:P, :], op=OP.subtract
    )
    for i in range(2):
        t2 = sp.tile([P, N], F32, tag=f"{tag}_t{i}")
        nc.vector.tensor_tensor(t2[:P, :], y[:P, :], y[:P, :], op=OP.mult)
        nc.vector.tensor_tensor(t2[:P, :], t2[:P, :], q, op=OP.mult)
        nc.vector.tensor_scalar(
            t2[:P, :], t2[:P, :], -0.5, 1.5, op0=OP.mult, op1=OP.add
        )
        nc.vector.tensor_tensor(y[:P, :], y[:P, :], t2[:P, :], op=OP.mult)
    return y


def emit_creation(nc, pools, consts, xT_ap, g, uh, uhT, box):
    (xp, up, utp, sp, ctp, pcre, pps, ptb, psm) = pools
    (W_t, dmask_t, vmask_t, identH_t, identF_t, ind4_t, ind4T_t, magic_t) = consts

    # ---- load xT for 4 examples: [D, (e, S)] as 2 partition tiles ----
    xt = []
    for k in range(KT_D):
        t = xp.tile([128, G, S], F32R, tag=f"xt{k}")
        nc.sync.dma_start(
            t[:],
            xT_ap[G * g : G * (g + 1), 128 * k : 128 * (k + 1), :].rearrange(
                "e p s -> p e s"
            ),
        )
        xt.append(t)

    # ---- u_hatT [O, S] per example (copies alternate ACT/DVE), and
    # ---- u_hat via PE transposes interleaved between matmul bursts ----
    acc = sp.tile([128, NC_], F32, tag="acc")

    def emit_uht(j):
        for t in range(MT):
            pu = pcre.tile([128, S], F32, tag="pcre")
            for k in range(KT_D):
                nc.tensor.matmul(
                    pu[:],
                    W_t[k][:, bass.ts(t, 128)],
                    xt[k][:, j, :],
                    start=(k == 0),
                    stop=(k == KT_D - 1),
                )
            ut = utp.tile([128, S], F16, tag=f"uht{j}{t}")
            a_col = acc[:, 4 * j + t : 4 * j + t + 1]
            if (j + t) % 2 == 0:
                nc.scalar.activation(ut[:], pu[:], AF.Copy, accum_out=a_col)
            else:
                nc.any.tensor_scalar(
                    ut[:], pu[:], 1.0, None, op0=OP.mult, op1=OP.add,
                    accum_out=a_col,
                )
            uhT[j][t] = ut

    def emit_uh(j):
        u = up.tile([128, MT, O], F16, tag=f"uh{j}")
        for q in range(2):
            tp = ptb.tile([128, 2 * O], F16, tag="tp")
            for mm in range(2):
                m = 2 * q + mm
                for t in range(MT):
                    nc.tensor.transpose(
                        tp[:, O * mm + 128 * t : O * mm + 128 * (t + 1)],
                        uhT[j][t][:, bass.ts(m, 128)],
                        identH_t[:],
                    )
            nc.vector.tensor_copy(
                u[:, 2 * q : 2 * q + 2, :].rearrange("p a b -> p (a b)"), tp[:]
            )
        for m in range(MT):
            uh[j][m] = u[:, m, :]

    emit_uht(0)
    yield
    for j in range(1, G):
        emit_uht(j)
        yield
        emit_uh(j - 1)
        yield
    emit_uh(G - 1)
    yield

    # ---- fused iteration 0: v0 from colsums, in O-partition layout ----
    # s0 = acc/16; |s0|^2 per capsule via 32-partition-group-sum matmul
    sqa = sp.tile([128, NC_], F32, tag="sqa")
    nc.vector.tensor_tensor(sqa[:], acc[:], acc[:], op=OP.mult)
    pnt = psm.tile([128, 1024], F16, tag="small")
    pn = pnt[:, 512:832].bitcast(F32)  # [128, 160] f32 window
    nc.tensor.matmul(pn[:4, 128:144], ind4_t[:], sqa[:], start=True, stop=True)
    q0 = sp.tile([4, NC_], F32, tag="q0")
    nc.vector.tensor_scalar(
        q0[:], pn[:4, 128:144], 1.0 / 256.0, 1e-7, op0=OP.mult, op1=OP.add
    )
    # f0 = rsqrt(q0)/16 ; v0 = acc * f0  (since v0 = (acc/16)*rsqrt(q0))
    y0 = quake_rsqrt(nc, sp, q0[:], magic_t, 4, NC_, "q0")
    f0 = sp.tile([4, NC_], F32, tag="f0")
    nc.vector.tensor_scalar_mul(f0[:], y0[:4, :], 1.0 / 16.0)
    nc.tensor.matmul(pn[:, 144:160], ind4T_t[:4, :], f0[:], start=True, stop=True)
    vv0 = sp.tile([128, NC_], F16, tag="vv0")
    nc.vector.tensor_tensor(vv0[:], acc[:], pn[:, 144:160], op=OP.mult)
    vblk0 = sp.tile([128, G * 4 * DC], F16, tag="vblk")
    nc.gpsimd.tensor_mul(
        vblk0[:].rearrange("p (j k n) -> p j k n", j=G, k=4),
        vmask_t[:].rearrange("p (j k n) -> p j k n", j=G, k=4),
        vv0[:]
        .rearrange("p (j k one) -> p j k one", j=G, one=1)
        .to_broadcast([128, G, 4, DC]),
    )
    box[0] = vblk0
    yield


def emit_routing(nc, pools, consts, out_ap, g, uh, uhT, box):
    (xp, up, utp, sp, ctp, pcre, pps, ptb, psm) = pools
    (W_t, dmask_t, vmask_t, identH_t, identF_t, ind4_t, ind4T_t, magic_t) = consts

    vblk = box[0]
    for it in range(2):
        # ---- b update: pb[32j+n, s] = v.u_hat, 4 examples in 4 col-groups --
        pb = pps.tile([128, S], F32, tag="ps")
        for j in range(G):
            for k in range(MT):
                nc.tensor.matmul(
                    pb[32 * j : 32 * j + 32, :],
                    vblk[:, 128 * j + DC * k : 128 * j + DC * (k + 1)],
                    uhT[j][k][:],
                    start=(k == 0),
                    stop=(k == MT - 1),
                    tile_position=(0, 32 * j),
                )
        yield
        expb = sp.tile([128, S], F16, tag="expb")
        nc.scalar.activation(expb[:], pb[:], AF.Exp)
        ett = psm.tile([128, 1024], F16, tag="small")
        et = ett[:, :512]
        for m in range(MT):
            nc.tensor.transpose(
                et[:, bass.ts(m, 128)], expb[:, bass.ts(m, 128)], identH_t[:]
            )
        # softmax over the 16 live columns of each 32-strip
        et_v = et[:].rearrange("p (m j n) -> p m j n", m=MT, j=G)[:, :, :, :NC_]
        r_all = sp.tile([128, MT * G], F32, tag="r_all")
        nc.vector.tensor_reduce(
            r_all[:].rearrange("p (m j) -> p m j", m=MT), et_v, axis=AX.X, op=OP.add
        )
        rinv = sp.tile([128, MT * G], F32, tag="rinv")
        nc.vector.reciprocal(rinv[:], r_all[:])
        ct = ctp.tile([128, MT * G * DC], F16, tag="ct")
        nc.vector.tensor_mul(
            ct[:].rearrange("p (m j n) -> p m j n", m=MT, j=G),
            et[:].rearrange("p (m j n) -> p m j n", m=MT, j=G),
            rinv[:]
            .rearrange("p (m j one) -> p m j one", m=MT, one=1)
            .to_broadcast([128, MT, G, DC]),
        )
        yield
        # ---- s matmul: 4 examples in 4 col-groups, accumulate over m ----
        ps = pps.tile([128, O], F32, tag="ps")
        for j in range(G):
            for m in range(MT):
                nc.tensor.matmul(
                    ps[32 * j : 32 * j + 32, :],
                    ct[:, 128 * m + DC * j : 128 * m + DC * (j + 1)],
                    uh[j][m][:],
                    start=(m == 0),
                    stop=(m == MT - 1),
                    tile_position=(0, 32 * j),
                )
        yield
        # ---- extract block-diagonal -> s [strip, d], then squash ----
        masked = sp.tile([128, O], F32, tag="masked")
        nc.vector.tensor_mul(masked[:], ps[:], dmask_t[:])
        s = sp.tile([128, DC], F32, tag="s")
        nc.vector.tensor_reduce(
            s[:],
            masked[:].rearrange("p (n d) -> p d n", n=NC_),
            axis=AX.X,
            op=OP.add,
        )
        sq2 = sp.tile([128, DC], F32, tag="sq2")
        nc.vector.tensor_tensor(sq2[:], s[:], s[:], op=OP.mult)
        q2 = sp.tile([128, 1], F32, tag="q2")
        nc.vector.tensor_reduce(q2[:], sq2[:], axis=AX.X, op=OP.add)
        nc.vector.tensor_scalar_add(q2[:], q2[:], 1e-7)
        y = quake_rsqrt(nc, sp, q2[:], magic_t, 128, 1, "q2")
        v = sp.tile([128, DC], F32, tag="v")
        nc.vector.tensor_scalar_mul(v[:], s[:], y[:])

        if it == 0:
            # ---- rebuild vblk from v (strip layout -> O layout) ----
            pvt = psm.tile([128, 1024], F16, tag="small")
            pv = pvt[:, 512:832].bitcast(F32)
            nc.tensor.transpose(pv[:DC, :128], v[:], identF_t[:])
            vv = sp.tile([128, NC_], F16, tag="vv")
            vtp_jx = pv[:DC, :128].rearrange("p (j x) -> p j x", j=G)
            for r in range(4):
                nc.vector.tensor_copy(
                    vv[32 * r : 32 * (r + 1), :].rearrange(
                        "p (j k) -> p j k", j=G
                    ),
                    vtp_jx[:, :, r : NC_ : 4],
                )
            vblk = sp.tile([128, G * 4 * DC], F16, tag="vblk")
            nc.gpsimd.tensor_mul(
                vblk[:].rearrange("p (j k n) -> p j k n", j=G, k=4),
                vmask_t[:].rearrange("p (j k n) -> p j k n", j=G, k=4),
                vv[:]
                .rearrange("p (j k one) -> p j k one", j=G, one=1)
                .to_broadcast([128, G, 4, DC]),
            )
            yield

    # ---- output: strip j -> row 4g+j ----
    for j in range(G):
        nc.sync.dma_start(
            out_ap[G * g + j].rearrange("(n d) -> n d", n=NC_),
            v[32 * j : 32 * j + NC_, :],
        )


def build(n_ex=E, num_devices=N_CORES):
    assert n_ex % G == 0
    nc = bacc.Bacc(
        "TRN2", target_bir_lowering=False, debug=False, num_devices=num_devices
    )
    xT_d = nc.dram_tensor("xT", [n_ex, D, S], F32R, kind="ExternalInput")
    W_d = nc.dram_tensor("W", [D, O], F32R, kind="ExternalInput")
    dmask_d = nc.dram_tensor("dmask", [128, O], F32, kind="ExternalInput")
    vmask_d = nc.dram_tensor("vmask", [128, G * 4 * DC], F16, kind="ExternalInput")
    identH_d = nc.dram_tensor("identH", [128, 128], F16, kind="ExternalInput")
    identF_d = nc.dram_tensor("identF", [128, 128], F32, kind="ExternalInput")
    ind4_d = nc.dram_tensor("ind4", [128, 4], F32, kind="ExternalInput")
    ind4T_d = nc.dram_tensor("ind4T", [4, 128], F32, kind="ExternalInput")
    out_d = nc.dram_tensor("out", [n_ex, O], F32, kind="ExternalOutput")

    with tile.TileContext(nc) as tc:
        with (
            tc.tile_pool(name="consts", bufs=1) as cp,
            tc.tile_pool(name="xp", bufs=3) as xp,
            tc.tile_pool(name="up", bufs=3) as up,
            tc.tile_pool(name="utp", bufs=3) as utp,
            tc.tile_pool(name="sp", bufs=3) as sp,
            tc.tile_pool(name="ctp", bufs=3) as ctp,
            tc.tile_pool(name="pcre", bufs=2, space=bass.MemorySpace.PSUM) as pcre,
            tc.tile_pool(name="pps", bufs=3, space=bass.MemorySpace.PSUM) as pps,
            tc.tile_pool(name="ptb", bufs=2, space=bass.MemorySpace.PSUM) as ptb,
            tc.tile_pool(name="psm", bufs=1, space=bass.MemorySpace.PSUM) as psm,
        ):
            W_t = []
            for k in range(KT_D):
                t = cp.tile([128, O], F32R, tag=f"W{k}")
                nc.sync.dma_start(t[:], W_d.ap()[128 * k : 128 * (k + 1), :])
                W_t.append(t)
            dmask_t = cp.tile([128, O], F32, tag="dmask")
            nc.sync.dma_start(dmask_t[:], dmask_d.ap())
            vmask_t = cp.tile([128, G * 4 * DC], F16, tag="vmask")
            nc.sync.dma_start(vmask_t[:], vmask_d.ap())
            identH_t = cp.tile([128, 128], F16, tag="identH")
            nc.sync.dma_start(identH_t[:], identH_d.ap())
            identF_t = cp.tile([128, 128], F32, tag="identF")
            nc.sync.dma_start(identF_t[:], identF_d.ap())
            ind4_t = cp.tile([128, 4], F32, tag="ind4")
            nc.sync.dma_start(ind4_t[:], ind4_d.ap())
            ind4T_t = cp.tile([4, 128], F32, tag="ind4T")
            nc.sync.dma_start(ind4T_t[:4, :], ind4T_d.ap())
            magic_t = cp.tile([128, NC_], U32, tag="magic")
            nc.vector.memset(magic_t[:], QMAGIC)

            pools = (xp, up, utp, sp, ctp, pcre, pps, ptb, psm)
            consts = (
                W_t, dmask_t, vmask_t, identH_t, identF_t, ind4_t, ind4T_t, magic_t
            )
            ngroups = n_ex // G

            def creation_gen(g):
                uh = [[None] * MT for _ in range(G)]
                uhT = [[None] * MT for _ in range(G)]
                box = [None]
                gen = emit_creation(nc, pools, consts, xT_d.ap(), g, uh, uhT, box)
                return gen, (uh, uhT, box)

            # Software pipeline: up to two groups' routing chains interleaved
            # with the next group's creation, so the PE always has work and
            # HAM stays warm.
            _stop = object()
            cgen, made = creation_gen(0)
            for _ in cgen:
                pass
            active = [emit_routing(nc, pools, consts, out_d.ap(), 0, *made)]
            if ngroups > 1:
                cgen, cmade = creation_gen(1)
                cgroup = 1
                next_create = 2
            else:
                cgen = None
            while active or cgen is not None:
                for rg in list(active):
                    if next(rg, _stop) is _stop:
                        active.remove(rg)
                if cgen is not None:
                    if next(cgen, _stop) is _stop:
                        active.append(
                            emit_routing(
                                nc, pools, consts, out_d.ap(), cgroup, *cmade
                            )
                        )
                        if next_create < ngroups:
                            cgen, cmade = creation_gen(next_create)
                            cgroup = next_create
                            next_create += 1
                        else:
                            cgen = None

    nc.compile()
    return nc


_cache = {}


def _get_program():
    if "nc" not in _cache:
        _cache["nc"] = build()
    return _cache["nc"]


def _run(x: np.ndarray, W: np.ndarray, **spmd_kwargs):
    x = np.asarray(x, np.float32)
    W = np.asarray(W, np.float32)
    nc = _get_program()
    xT = np.ascontiguousarray(x.transpose(0, 2, 1))  # [B, D, S]
    dmask, vmask, identH, identF, ind4, ind4T = host_consts()
    in_maps = []
    for c in range(N_CORES):
        in_maps.append(
            {
                "xT": xT[c * E : (c + 1) * E],
                "W": W,
                "dmask": dmask,
                "vmask": vmask,
                "identH": identH,
                "identF": identF,
                "ind4": ind4,
                "ind4T": ind4T,
            }
        )
    res = run_bass_kernel_spmd(
        nc, in_maps, core_ids=list(range(N_CORES)), **spmd_kwargs
    )
    out = np.concatenate([res.results[c]["out"] for c in range(N_CORES)], axis=0)
    return out, res


def kernel(x: np.ndarray, W: np.ndarray) -> np.ndarray:
    return _run(x, W)[0]
